# revision 17
# baseline (speedup 1.0000x reference)
"""Multi-head attention (B=2, L=2048, D=2048, H=16, causal + RoPE) on 8 TRN2 cores.

Sharding: tensor-parallel over heads. Core c owns heads {2c, 2c+1}:
  - wq/wk/wv column slices [D, 256], wo row slice [256, D]
  - each core computes a partial output y_c = att_c @ wo_c  (full shape)
  - host reduces: y = sum_c y_c   (the "all-reduce" of the output projection)

Device kernel (per core, SPMD):
  1. Projections: QT/KT = (w^T x^T) in transposed layout [head_dim, tok],
     V in natural layout [tok, head_dim]; RoPE applied to QT/KT rows 0:64
     per head (head-dim pre-permuted on host so RoPE pairs are (i, i+32)).
  2. Attention per (batch, head): causal, no-max-subtraction softmax
     (scores ~ N(0,1), exp is safe in fp32):
       ST_ij = K_j^T Q_i   [tok_j=128, tok_i=512]  (PE, fp32r)
       ET    = exp(ST)     (ACT) ; diagonal quarters masked via mask mult
       r_i  += ones^T ET   [1, 512]   (PE)   -- softmax denominators
       avT  += V_j^T ET    [hd=128, tok_i=512] (PE, accumulated over j)
  3. Output: y[tok, :] = sum_h (att_h^T)^T wo_h / r_h  -- per-head PSUM
     results scaled by 1/r (per-partition scalar) and added.

All matmuls run as float32r (FP22 multiply, fp32 accumulate): 1 cycle/row
on the PE for moving dim >= 256, 4x faster than true fp32.
"""

import glob
import os
import sys


def _ensure_env():
    # walrus_driver (neuronx-cc) must be on PATH for client-side NEFF compile.
    if not any("-b16-bazel-" in p for p in os.environ.get("PATH", "").split(":")):
        cands = sorted(glob.glob("/nix/store/*-b16-bazel-*/bin"))
        for c in cands:
            if os.path.exists(os.path.join(c, "neuronx-cc")) or glob.glob(
                os.path.join(c, "*walrus*")
            ):
                os.environ["PATH"] = c + ":" + os.environ["PATH"]
                break
        else:
            if cands:
                os.environ["PATH"] = cands[-1] + ":" + os.environ["PATH"]


_ensure_env()
os.environ.setdefault("JAX_COMPILATION_CACHE_DIR", "/tmp/jax_comp_cache")
os.environ.setdefault("JAX_PERSISTENT_CACHE_MIN_COMPILE_TIME_SECS", "1")
os.environ.setdefault("JAX_PERSISTENT_CACHE_MIN_ENTRY_SIZE_BYTES", "0")

import numpy as np  # noqa: E402

import concourse.bass as bass  # noqa: E402
import concourse.mybir as mybir  # noqa: E402
import concourse.tile as tile  # noqa: E402
from concourse import bacc  # noqa: E402
from concourse.bass_utils import run_bass_kernel_spmd  # noqa: E402

NCORES = 8
B, L, D = 2, 2048, 2048
H = 16
HD = 128            # head dim
HPC = H // NCORES   # heads per core
DQ = HPC * HD       # 256: per-core projection width
ROPE = 64           # RoPE dims per head
F32 = mybir.dt.float32
F32R = mybir.dt.float32r

NT256 = L // 256    # 8 token tiles for projections
NI = L // 512       # 4 i-tiles per attention instance
NJ = L // 128       # 16 j-blocks


def _r(ap):
    """fp32 -> fp32r view for PE operands (unused: tiles are f32r natively)."""
    return ap.bitcast(F32R)


def build_nc():
    nc = bacc.Bacc(
        "TRN2", target_bir_lowering=False, debug=False, num_devices=NCORES
    )
    BF16 = mybir.dt.bfloat16
    xt = nc.dram_tensor("xt", [B, D, L], F32R, kind="ExternalInput").ap()
    wq = nc.dram_tensor("wq", [D, DQ], F32R, kind="ExternalInput").ap()
    wk = nc.dram_tensor("wk", [D, DQ], F32R, kind="ExternalInput").ap()
    wv = nc.dram_tensor("wv", [D, DQ], F32R, kind="ExternalInput").ap()
    # "swapped" projections for RoPE: rows 0:64 of wqb^T x^T give [-x2; x1]
    wqb = nc.dram_tensor("wqb", [D, HPC * ROPE], F32R, kind="ExternalInput").ap()
    wkb = nc.dram_tensor("wkb", [D, HPC * ROPE], F32R, kind="ExternalInput").ap()
    wo = nc.dram_tensor("wo", [DQ, D], F32R, kind="ExternalInput").ap()
    # cs rows 0:64 = [cos;cos], rows 64:128 = [sin;sin] (bf16)
    cs = nc.dram_tensor("cs", [2 * ROPE, L], BF16, kind="ExternalInput").ap()
    m4 = nc.dram_tensor("m4", [128, 4, 512], BF16, kind="ExternalInput").ap()
    y = nc.dram_tensor("y", [B, L, D], F32, kind="ExternalOutput").ap()
    # scratch for transposing softmax denominators [1, L] -> [128, L/128]
    rb = nc.dram_tensor("rb", [B, HPC, L], F32, kind="Internal").ap()

    with tile.TileContext(nc) as tc:
        with (
            tc.tile_pool(name="consts", bufs=1) as consts,
            tc.tile_pool(name="wpool", bufs=1) as wpool,
            tc.tile_pool(name="qkv", bufs=1) as qkv,
            tc.tile_pool(name="xc", bufs=2) as xcpool,
            tc.tile_pool(name="et", bufs=3) as etpool,
            tc.tile_pool(name="rope", bufs=4) as ropepool,
            tc.tile_pool(name="ysb", bufs=4) as ypool,
            tc.tile_pool(name="rt", bufs=2) as rtpool,
            tc.tile_pool(name="rsb", bufs=2) as rsbpool,
            tc.tile_pool(name="pA", bufs=2, space="PSUM") as pA,
            tc.tile_pool(name="pST", bufs=2, space="PSUM") as pST,
            tc.tile_pool(name="pAV", bufs=2, space="PSUM") as pAV,
            tc.tile_pool(name="pR", bufs=2, space="PSUM") as pR,
        ):
            # ---- constants / weights ----
            ones_f = consts.tile([128, 1], F32)
            nc.vector.memset(ones_f, 1.0)
            ones = consts.tile([128, 1], F32R)
            nc.vector.tensor_copy(ones, ones_f)
            cs_sb = consts.tile([2 * ROPE, L], BF16)
            nc.sync.dma_start(out=cs_sb, in_=cs)
            m4_sb = consts.tile([128, 4, 512], BF16)
            nc.sync.dma_start(out=m4_sb, in_=m4)

            wq_sb = wpool.tile([128, 16, DQ], F32R)
            wk_sb = wpool.tile([128, 16, DQ], F32R)
            wv_sb = wpool.tile([128, 16, DQ], F32R)
            wqb_sb = wpool.tile([128, 16, HPC * ROPE], F32R)
            wkb_sb = wpool.tile([128, 16, HPC * ROPE], F32R)
            for w_dram, w_sb in (
                (wq, wq_sb),
                (wk, wk_sb),
                (wv, wv_sb),
                (wqb, wqb_sb),
                (wkb, wkb_sb),
            ):
                for qtr in range(4):
                    nc.sync.dma_start(
                        out=w_sb[:, 4 * qtr : 4 * qtr + 4, :],
                        in_=w_dram[512 * qtr : 512 * qtr + 512, :].rearrange(
                            "(c p) o -> p c o", p=128
                        ),
                    )
            wo_sb = wpool.tile([128, HPC, D], F32R)
            for h in range(HPC):
                nc.sync.dma_start(
                    out=wo_sb[:, h, :], in_=wo[HD * h : HD * h + HD, :]
                )

            qt_sb = qkv.tile([128, HPC, L], F32R)   # [d, h, tok]
            kt_sb = qkv.tile([128, HPC, L], F32R)
            v_sb = qkv.tile([128, NJ, DQ], F32R)    # [tok_in_blk, jblk, hd]
            att_sb = qkv.tile([128, HPC, L], F32R)  # [hd, h, tok] unnormalized
            rec_sb = qkv.tile([128, HPC, NJ], F32)  # 1/r  [tok128, h, toktile]

            for b in range(B):
                # ---------- projections ----------
                for tt in range(NT256):
                    xc = xcpool.tile([128, 16, 256], F32R, tag="xc")
                    for half in range(2):
                        nc.sync.dma_start(
                            out=xc[:, 8 * half : 8 * half + 8, :],
                            in_=xt[
                                b,
                                1024 * half : 1024 * half + 1024,
                                256 * tt : 256 * tt + 256,
                            ].rearrange("(c p) t -> p c t", p=128),
                        )
                    # QT / KT rows (transposed layout) + RoPE
                    for w_sb, wb_sb, out_sb in (
                        (wq_sb, wqb_sb, qt_sb),
                        (wk_sb, wkb_sb, kt_sb),
                    ):
                        for rt in range(HPC):
                            pp = pA.tile([128, 512], F32, tag="pA")
                            pj = pp[:, 0:256]
                            for c in range(16):
                                nc.tensor.matmul(
                                    pj,
                                    (w_sb[:, c, 128 * rt : 128 * rt + 128]),
                                    (xc[:, c, :]),
                                    start=(c == 0),
                                    stop=(c == 15),
                                )
                            # swapped projection rows [-x2; x1]
                            pb = pR.tile([ROPE, 512], F32, tag="pR")
                            pbj = pb[:, 0:256]
                            for c in range(16):
                                nc.tensor.matmul(
                                    pbj,
                                    (wb_sb[:, c, ROPE * rt : ROPE * rt + ROPE]),
                                    (xc[:, c, :]),
                                    start=(c == 0),
                                    stop=(c == 15),
                                )
                            dst = out_sb[:, rt, 256 * tt : 256 * tt + 256]
                            ccl = cs_sb[0:ROPE, 256 * tt : 256 * tt + 256]
                            ssl = cs_sb[ROPE:, 256 * tt : 256 * tt + 256]
                            t1 = ropepool.tile([ROPE, 256], F32, tag="rope")
                            t2 = ropepool.tile([ROPE, 256], F32, tag="rope")
                            # rot = cos*[x1;x2] + sin*[-x2;x1]
                            nc.vector.tensor_mul(t1, pj[0:ROPE], ccl)
                            nc.vector.tensor_mul(t2, pbj, ssl)
                            nc.vector.tensor_add(dst[0:ROPE], t1, t2)
                            nc.vector.tensor_copy(dst[ROPE:128], pj[ROPE:128])
                    # V rows (natural layout)
                    for ts2 in range(2):
                        pv = pA.tile([128, 512], F32, tag="pA")
                        pvj = pv[:, 0:256]
                        for c in range(16):
                            nc.tensor.matmul(
                                pvj,
                                (xc[:, c, 128 * ts2 : 128 * ts2 + 128]),
                                (wv_sb[:, c, :]),
                                start=(c == 0),
                                stop=(c == 15),
                            )
                        nc.vector.tensor_copy(v_sb[:, 2 * tt + ts2, :], pvj)

                # ---------- attention ----------
                for h in range(HPC):
                    for t in range(NI):
                        av = pAV.tile([128, 512], F32, tag="pAV")
                        rp = pR.tile([1, 512], F32, tag="pR")
                        njb = 4 * t + 4
                        for j in range(njb):
                            st = pST.tile([128, 512], F32, tag="pST")
                            nc.tensor.matmul(
                                st,
                                (kt_sb[:, h, 128 * j : 128 * j + 128]),
                                (qt_sb[:, h, 512 * t : 512 * t + 512]),
                                start=True,
                                stop=True,
                            )
                            et = etpool.tile([128, 512], F32R, tag="et")
                            nc.scalar.activation(
                                et, st, mybir.ActivationFunctionType.Exp
                            )
                            q = j - 4 * t
                            if q >= 0:
                                nc.vector.tensor_mul(et, et, m4_sb[:, q, :])
                            nc.tensor.matmul(
                                rp,
                                (ones),
                                (et),
                                start=(j == 0),
                                stop=(j == njb - 1),
                            )
                            nc.tensor.matmul(
                                av,
                                (v_sb[:, j, HD * h : HD * h + HD]),
                                (et),
                                start=(j == 0),
                                stop=(j == njb - 1),
                            )
                        nc.vector.tensor_copy(
                            att_sb[:, h, 512 * t : 512 * t + 512], av
                        )
                        r_sb = rsbpool.tile([1, 512], F32, tag="rsb")
                        nc.scalar.copy(r_sb, rp)
                        nc.sync.dma_start(
                            out=rb[b, h, 512 * t : 512 * t + 512], in_=r_sb
                        )
                # denominators: [1, L] -> [128, L/128] transpose via DRAM bounce
                rt_sb = rtpool.tile([128, HPC, NJ], F32, tag="rt")
                nc.sync.dma_start(
                    out=rt_sb,
                    in_=rb[b].rearrange("h (t p) -> p h t", p=128),
                )
                nc.vector.reciprocal(rec_sb[:, :, :], rt_sb)

                # ---------- output projection (partial sum over local heads) ----------
                for t2 in range(NJ):
                    for dd in range(4):
                        p0 = pST.tile([128, 512], F32, tag="pST")
                        p1 = pAV.tile([128, 512], F32, tag="pAV")
                        nc.tensor.matmul(
                            p0,
                            (att_sb[:, 0, 128 * t2 : 128 * t2 + 128]),
                            (wo_sb[:, 0, 512 * dd : 512 * dd + 512]),
                            start=True,
                            stop=True,
                        )
                        nc.tensor.matmul(
                            p1,
                            (att_sb[:, 1, 128 * t2 : 128 * t2 + 128]),
                            (wo_sb[:, 1, 512 * dd : 512 * dd + 512]),
                            start=True,
                            stop=True,
                        )
                        tmp = ypool.tile([128, 512], F32, tag="ysb")
                        nc.scalar.activation(
                            tmp,
                            p1,
                            mybir.ActivationFunctionType.Copy,
                            scale=rec_sb[:, 1, t2 : t2 + 1],
                        )
                        yt = ypool.tile([128, 512], F32, tag="ysb")
                        nc.vector.scalar_tensor_tensor(
                            yt,
                            p0,
                            rec_sb[:, 0, t2 : t2 + 1],
                            tmp,
                            op0=mybir.AluOpType.mult,
                            op1=mybir.AluOpType.add,
                        )
                        nc.sync.dma_start(
                            out=y[
                                b,
                                128 * t2 : 128 * t2 + 128,
                                512 * dd : 512 * dd + 512,
                            ],
                            in_=yt,
                        )
    nc.compile()
    return nc


_NC = None


def _get_nc():
    global _NC
    if _NC is None:
        _NC = build_nc()
    return _NC


def _host_inputs(x, mask, wq, wk, wv, wo):
    x = np.asarray(x, np.float32)
    wq = np.asarray(wq, np.float32)
    wk = np.asarray(wk, np.float32)
    wv = np.asarray(wv, np.float32)
    wo = np.asarray(wo, np.float32)
    mask = np.asarray(mask)

    xt = np.ascontiguousarray(x.transpose(0, 2, 1))  # [B, D, L]

    import ml_dtypes

    # permute head dims so RoPE pairs are (i, i+32): [evens, odds, pass-through]
    perm128 = np.concatenate(
        [np.arange(0, ROPE, 2), np.arange(1, ROPE, 2), np.arange(ROPE, HD)]
    )
    permD = np.concatenate([h * HD + perm128 for h in range(H)])
    wq_p = (wq * np.float32(1.0 / np.sqrt(HD)))[:, permD]
    wk_p = wk[:, permD]

    # "swapped" RoPE projections: per head 64 cols giving rows [-x2; x1]
    def swapped(w_p):
        wb = np.empty((D, H * ROPE), np.float32)
        for h in range(H):
            src = w_p[:, h * HD : h * HD + HD]
            wb[:, h * ROPE : h * ROPE + 32] = -src[:, 32:64]
            wb[:, h * ROPE + 32 : h * ROPE + ROPE] = src[:, 0:32]
        return wb

    wqb = swapped(wq_p)
    wkb = swapped(wk_p)

    # RoPE tables, matching reference fp32 math (dim=64, repeat-2 interleave)
    ts_ = np.arange(0, ROPE, 2, dtype=np.float32)
    inv = (np.float32(10000.0) ** (-ts_ / np.float32(ROPE))).astype(np.float32)
    grid = np.arange(L, dtype=np.float32)[:, None] * inv[None, :]  # [L, 32]
    cs = np.empty((2 * ROPE, L), np.float32)
    cs[0:32] = cs[32:64] = np.cos(grid).T
    cs[64:96] = cs[96:128] = np.sin(grid).T
    cs = cs.astype(ml_dtypes.bfloat16)

    # diagonal-quarter masks for ST tiles: m4[j, q, i] = mask[i, 128q + j]
    mm = np.asarray(mask[0, 0, :512, :512])
    m4 = (
        mm.T.reshape(4, 128, 512).transpose(1, 0, 2).astype(ml_dtypes.bfloat16)
    )  # [j, q, i]

    in_maps = []
    for c in range(NCORES):
        sl = slice(DQ * c, DQ * c + DQ)
        slb = slice(HPC * ROPE * c, HPC * ROPE * (c + 1))
        in_maps.append(
            {
                "xt": xt,
                "wq": np.ascontiguousarray(wq_p[:, sl]),
                "wk": np.ascontiguousarray(wk_p[:, sl]),
                "wv": np.ascontiguousarray(wv[:, sl]),
                "wqb": np.ascontiguousarray(wqb[:, slb]),
                "wkb": np.ascontiguousarray(wkb[:, slb]),
                "wo": np.ascontiguousarray(wo[sl, :]),
                "cs": cs,
                "m4": m4,
            }
        )
    return in_maps


def kernel(**inputs):
    nc = _get_nc()
    in_maps = _host_inputs(
        inputs["x"], inputs["mask"], inputs["wq"], inputs["wk"],
        inputs["wv"], inputs["wo"],
    )
    res = run_bass_kernel_spmd(nc, in_maps, core_ids=list(range(NCORES)))
    out = res.results[0]["y"].astype(np.float64)
    for c in range(1, NCORES):
        out += res.results[c]["y"]
    return out.astype(np.float32)


# revision 28
# speedup vs baseline: 1.0648x; 1.0648x over previous
"""Multi-head attention (B=2, L=2048, D=2048, H=16, causal + RoPE) on 8 TRN2 cores.

Sharding: tensor-parallel over heads. Core c owns heads {2c, 2c+1}:
  - wq/wk/wv column slices [D, 256], wo row slice [256, D]
  - each core computes a partial output y_c = att_c @ wo_c  (full shape)
  - host reduces: y = sum_c y_c   (the "all-reduce" of the output projection)

Device kernel (per core, SPMD):
  1. Projections: QT/KT = (w^T x^T) in transposed layout [head_dim, tok],
     V in natural layout [tok, head_dim]; RoPE applied to QT/KT rows 0:64
     per head (head-dim pre-permuted on host so RoPE pairs are (i, i+32)).
  2. Attention per (batch, head): causal, no-max-subtraction softmax
     (scores ~ N(0,1), exp is safe in fp32):
       ST_ij = K_j^T Q_i   [tok_j=128, tok_i=512]  (PE, fp32r)
       ET    = exp(ST)     (ACT) ; diagonal quarters masked via mask mult
       r_i  += ones^T ET   [1, 512]   (PE)   -- softmax denominators
       avT  += V_j^T ET    [hd=128, tok_i=512] (PE, accumulated over j)
  3. Output: y[tok, :] = sum_h (att_h^T)^T wo_h / r_h  -- per-head PSUM
     results scaled by 1/r (per-partition scalar) and added.

All matmuls run as float32r (FP22 multiply, fp32 accumulate): 1 cycle/row
on the PE for moving dim >= 256, 4x faster than true fp32.
"""

import glob
import os
import sys


def _ensure_env():
    # walrus_driver (neuronx-cc) must be on PATH for client-side NEFF compile.
    if not any("-b16-bazel-" in p for p in os.environ.get("PATH", "").split(":")):
        cands = sorted(glob.glob("/nix/store/*-b16-bazel-*/bin"))
        for c in cands:
            if os.path.exists(os.path.join(c, "neuronx-cc")) or glob.glob(
                os.path.join(c, "*walrus*")
            ):
                os.environ["PATH"] = c + ":" + os.environ["PATH"]
                break
        else:
            if cands:
                os.environ["PATH"] = cands[-1] + ":" + os.environ["PATH"]


_ensure_env()
os.environ.setdefault("JAX_COMPILATION_CACHE_DIR", "/tmp/jax_comp_cache")
os.environ.setdefault("JAX_PERSISTENT_CACHE_MIN_COMPILE_TIME_SECS", "1")
os.environ.setdefault("JAX_PERSISTENT_CACHE_MIN_ENTRY_SIZE_BYTES", "0")

import numpy as np  # noqa: E402

import concourse.bass as bass  # noqa: E402
import concourse.mybir as mybir  # noqa: E402
import concourse.tile as tile  # noqa: E402
from concourse import bacc  # noqa: E402
from concourse.bass_utils import run_bass_kernel_spmd  # noqa: E402

NCORES = 8
B, L, D = 2, 2048, 2048
H = 16
HD = 128            # head dim
HPC = H // NCORES   # heads per core
DQ = HPC * HD       # 256: per-core projection width
ROPE = 64           # RoPE dims per head
F32 = mybir.dt.float32
F32R = mybir.dt.float32r

NT256 = L // 256    # 8 token tiles for projections
NI = L // 512       # 4 i-tiles per attention instance
NJ = L // 128       # 16 j-blocks


def _r(ap):
    """fp32 -> fp32r view for PE operands (unused: tiles are f32r natively)."""
    return ap.bitcast(F32R)


def build_nc():
    nc = bacc.Bacc(
        "TRN2", target_bir_lowering=False, debug=False, num_devices=NCORES
    )
    BF16 = mybir.dt.bfloat16
    xt = nc.dram_tensor("xt", [B, D, L], F32R, kind="ExternalInput").ap()
    wq = nc.dram_tensor("wq", [D, DQ], F32R, kind="ExternalInput").ap()
    wk = nc.dram_tensor("wk", [D, DQ], F32R, kind="ExternalInput").ap()
    wv = nc.dram_tensor("wv", [D, DQ], F32R, kind="ExternalInput").ap()
    wo = nc.dram_tensor("wo", [DQ, D], F32R, kind="ExternalInput").ap()
    # cc rows = [cos;cos], ss rows = [-sin;+sin] (bf16), for pairs (i, i+32)
    cc = nc.dram_tensor("cc", [ROPE, L], BF16, kind="ExternalInput").ap()
    ss = nc.dram_tensor("ss", [ROPE, L], BF16, kind="ExternalInput").ap()
    m4 = nc.dram_tensor("m4", [128, 4, 512], BF16, kind="ExternalInput").ap()
    y = nc.dram_tensor("y", [B, L, D], F32, kind="ExternalOutput").ap()
    # scratch for transposing softmax denominators [1, L] -> [128, L/128]
    rb = nc.dram_tensor("rb", [B, HPC, L], F32, kind="Internal").ap()

    with tile.TileContext(nc) as tc:
        with (
            tc.tile_pool(name="consts", bufs=1) as consts,
            tc.tile_pool(name="wpool", bufs=1) as wpool,
            tc.tile_pool(name="qkv", bufs=1) as qkv,
            tc.tile_pool(name="xc", bufs=2) as xcpool,
            tc.tile_pool(name="et", bufs=3) as etpool,
            tc.tile_pool(name="rope", bufs=2) as ropepool,
            tc.tile_pool(name="ysb", bufs=4) as ypool,
            tc.tile_pool(name="rt", bufs=2) as rtpool,
            tc.tile_pool(name="rsb", bufs=2) as rsbpool,
            tc.tile_pool(name="pA", bufs=2, space="PSUM") as pA,
            tc.tile_pool(name="pST", bufs=3, space="PSUM") as pST,
            tc.tile_pool(name="pAV", bufs=2, space="PSUM") as pAV,
            tc.tile_pool(name="pR", bufs=1, space="PSUM") as pR,
        ):
            # ---- constants / weights ----
            ones_f = consts.tile([128, 1], F32)
            nc.vector.memset(ones_f, 1.0)
            ones = consts.tile([128, 1], F32R)
            nc.vector.tensor_copy(ones, ones_f)
            cc_sb = consts.tile([ROPE, L], BF16)
            nc.sync.dma_start(out=cc_sb, in_=cc)
            ss_sb = consts.tile([ROPE, L], BF16)
            nc.sync.dma_start(out=ss_sb, in_=ss)
            m4_sb = consts.tile([128, 4, 512], BF16)
            nc.sync.dma_start(out=m4_sb, in_=m4)

            # preload the first token-chunk of x before the weight DMAs so
            # the first projection matmuls start as early as possible
            xc_pre = xcpool.tile([128, 16, 256], F32R, tag="xc")
            for oct_ in range(8):
                nc.sync.dma_start(
                    out=xc_pre[:, 2 * oct_ : 2 * oct_ + 2, :],
                    in_=xt[0, 256 * oct_ : 256 * oct_ + 256, 0:256].rearrange(
                        "(c p) t -> p c t", p=128
                    ),
                )

            wq_sb = wpool.tile([128, 16, DQ], F32R)
            wk_sb = wpool.tile([128, 16, DQ], F32R)
            wv_sb = wpool.tile([128, 16, DQ], F32R)
            for w_dram, w_sb in (
                (wq, wq_sb),
                (wk, wk_sb),
                (wv, wv_sb),
            ):
                for oct_ in range(8):
                    nc.sync.dma_start(
                        out=w_sb[:, 2 * oct_ : 2 * oct_ + 2, :],
                        in_=w_dram[256 * oct_ : 256 * oct_ + 256, :].rearrange(
                            "(c p) o -> p c o", p=128
                        ),
                    )
            wo_sb = wpool.tile([128, HPC, D], F32R)
            for h in range(HPC):
                nc.sync.dma_start(
                    out=wo_sb[:, h, :], in_=wo[HD * h : HD * h + HD, :]
                )

            qt_sb = qkv.tile([128, HPC, L], F32R)   # [d, h, tok]
            kt_sb = qkv.tile([128, HPC, L], F32R)
            v_sb = qkv.tile([128, NJ, DQ], F32R)    # [tok_in_blk, jblk, hd]
            att_sb = qkv.tile([128, HPC, L], F32R)  # [hd, h, tok] unnormalized
            rec_sb = qkv.tile([128, HPC, NJ], F32)  # 1/r  [tok128, h, toktile]

            for b in range(B):
                # ---------- projections ----------
                for tt in range(NT256):
                    if b == 0 and tt == 0:
                        xc = xc_pre
                    else:
                        xc = xcpool.tile([128, 16, 256], F32R, tag="xc")
                        for oct_ in range(8):
                            nc.sync.dma_start(
                                out=xc[:, 2 * oct_ : 2 * oct_ + 2, :],
                                in_=xt[
                                    b,
                                    256 * oct_ : 256 * oct_ + 256,
                                    256 * tt : 256 * tt + 256,
                                ].rearrange("(c p) t -> p c t", p=128),
                            )
                    # QT / KT rows (transposed layout); RoPE applied later
                    for w_sb, out_sb in (
                        (wq_sb, qt_sb),
                        (wk_sb, kt_sb),
                    ):
                        for rt in range(HPC):
                            pp = pA.tile([128, 512], F32, tag="pA")
                            pj = pp[:, 0:256]
                            for c in range(16):
                                nc.tensor.matmul(
                                    pj,
                                    (w_sb[:, c, 128 * rt : 128 * rt + 128]),
                                    (xc[:, c, :]),
                                    start=(c == 0),
                                    stop=(c == 15),
                                )
                            dst = out_sb[:, rt, 256 * tt : 256 * tt + 256]
                            nc.vector.tensor_copy(dst, pj)
                    # V rows (natural layout)
                    for ts2 in range(2):
                        pv = pA.tile([128, 512], F32, tag="pA")
                        pvj = pv[:, 0:256]
                        for c in range(16):
                            nc.tensor.matmul(
                                pvj,
                                (xc[:, c, 128 * ts2 : 128 * ts2 + 128]),
                                (wv_sb[:, c, :]),
                                start=(c == 0),
                                stop=(c == 15),
                            )
                        nc.vector.tensor_copy(v_sb[:, 2 * tt + ts2, :], pvj)

                # ---------- RoPE (wide, in place) ----------
                # swap = [x2; x1] via partition-shift DMAs, then
                # rot = [x1;x2]*[c;c] + [x2;x1]*[-s;s]
                for rt in range(HPC):
                    for out_sb in (qt_sb, kt_sb):
                        rope_rows = out_sb[0:ROPE, rt, :]
                        swap = ropepool.tile([ROPE, L], F32R, tag="rope")
                        for ch in range(4):
                            csl = slice(512 * ch, 512 * ch + 512)
                            nc.sync.dma_start(
                                out=swap[0:32, csl], in_=out_sb[32:64, rt, csl]
                            )
                            nc.sync.dma_start(
                                out=swap[32:64, csl], in_=out_sb[0:32, rt, csl]
                            )
                        nc.vector.tensor_mul(swap, swap, ss_sb)
                        nc.vector.tensor_mul(rope_rows, rope_rows, cc_sb)
                        nc.vector.tensor_add(rope_rows, rope_rows, swap)

                # ---------- attention ----------
                for h in range(HPC):
                    for t in range(NI):
                        av = pAV.tile([128, 512], F32, tag="pAV")
                        rp = pR.tile([1, 512], F32, tag="pR")
                        njb = 4 * t + 4
                        for j in range(njb):
                            st = pST.tile([128, 512], F32, tag="pST")
                            nc.tensor.matmul(
                                st,
                                (kt_sb[:, h, 128 * j : 128 * j + 128]),
                                (qt_sb[:, h, 512 * t : 512 * t + 512]),
                                start=True,
                                stop=True,
                            )
                            et = etpool.tile([128, 512], F32R, tag="et")
                            nc.scalar.activation(
                                et, st, mybir.ActivationFunctionType.Exp
                            )
                            q = j - 4 * t
                            if q >= 0:
                                nc.gpsimd.tensor_mul(et, et, m4_sb[:, q, :])
                            nc.tensor.matmul(
                                rp,
                                (ones),
                                (et),
                                start=(j == 0),
                                stop=(j == njb - 1),
                            )
                            nc.tensor.matmul(
                                av,
                                (v_sb[:, j, HD * h : HD * h + HD]),
                                (et),
                                start=(j == 0),
                                stop=(j == njb - 1),
                            )
                        nc.vector.tensor_copy(
                            att_sb[:, h, 512 * t : 512 * t + 512], av
                        )
                        r_sb = rsbpool.tile([1, 512], F32, tag="rsb")
                        nc.scalar.copy(r_sb, rp)
                        nc.sync.dma_start(
                            out=rb[b, h, 512 * t : 512 * t + 512], in_=r_sb
                        )
                # denominators: [1, L] -> [128, L/128] transpose via DRAM bounce
                rt_sb = rtpool.tile([128, HPC, NJ], F32, tag="rt")
                for h in range(HPC):
                    nc.sync.dma_start(
                        out=rt_sb[:, h, :],
                        in_=rb[b, h, :].rearrange("(t2 p) -> p t2", p=128),
                    )
                nc.vector.reciprocal(rec_sb[:, :, :], rt_sb)

                # ---------- output projection (partial sum over local heads) ----------
                for t2 in range(NJ):
                    for dd in range(4):
                        p0 = pST.tile([128, 512], F32, tag="pST")
                        p1 = pAV.tile([128, 512], F32, tag="pAV")
                        nc.tensor.matmul(
                            p0,
                            (att_sb[:, 0, 128 * t2 : 128 * t2 + 128]),
                            (wo_sb[:, 0, 512 * dd : 512 * dd + 512]),
                            start=True,
                            stop=True,
                        )
                        nc.tensor.matmul(
                            p1,
                            (att_sb[:, 1, 128 * t2 : 128 * t2 + 128]),
                            (wo_sb[:, 1, 512 * dd : 512 * dd + 512]),
                            start=True,
                            stop=True,
                        )
                        tmp = ypool.tile([128, 512], F32, tag="ysb")
                        nc.scalar.activation(
                            tmp,
                            p1,
                            mybir.ActivationFunctionType.Copy,
                            scale=rec_sb[:, 1, t2 : t2 + 1],
                        )
                        yt = ypool.tile([128, 512], F32, tag="ysb")
                        nc.vector.scalar_tensor_tensor(
                            yt,
                            p0,
                            rec_sb[:, 0, t2 : t2 + 1],
                            tmp,
                            op0=mybir.AluOpType.mult,
                            op1=mybir.AluOpType.add,
                        )
                        nc.sync.dma_start(
                            out=y[
                                b,
                                128 * t2 : 128 * t2 + 128,
                                512 * dd : 512 * dd + 512,
                            ],
                            in_=yt,
                        )
    nc.compile()
    return nc


_NC = None


def _get_nc():
    global _NC
    if _NC is None:
        _NC = build_nc()
    return _NC


def _host_inputs(x, mask, wq, wk, wv, wo):
    x = np.asarray(x, np.float32)
    wq = np.asarray(wq, np.float32)
    wk = np.asarray(wk, np.float32)
    wv = np.asarray(wv, np.float32)
    wo = np.asarray(wo, np.float32)
    mask = np.asarray(mask)

    xt = np.ascontiguousarray(x.transpose(0, 2, 1))  # [B, D, L]

    import ml_dtypes

    # permute head dims so RoPE pairs are (i, i+32): [evens, odds, pass-through]
    perm128 = np.concatenate(
        [np.arange(0, ROPE, 2), np.arange(1, ROPE, 2), np.arange(ROPE, HD)]
    )
    permD = np.concatenate([h * HD + perm128 for h in range(H)])
    wq_p = (wq * np.float32(1.0 / np.sqrt(HD)))[:, permD]
    wk_p = wk[:, permD]

    # RoPE tables, matching reference fp32 math (dim=64, repeat-2 interleave)
    # cc = [cos; cos], ss = [-sin; +sin] for the (x1=rows 0:32, x2=rows 32:64)
    # pairing: rot = [x1;x2]*cc + [x2;x1]*ss
    ts_ = np.arange(0, ROPE, 2, dtype=np.float32)
    inv = (np.float32(10000.0) ** (-ts_ / np.float32(ROPE))).astype(np.float32)
    grid = np.arange(L, dtype=np.float32)[:, None] * inv[None, :]  # [L, 32]
    cc = np.empty((ROPE, L), np.float32)
    cc[0:32] = cc[32:64] = np.cos(grid).T
    ss = np.empty((ROPE, L), np.float32)
    ss[0:32] = -np.sin(grid).T
    ss[32:64] = np.sin(grid).T
    cc = cc.astype(ml_dtypes.bfloat16)
    ss = ss.astype(ml_dtypes.bfloat16)

    # diagonal-quarter masks for ST tiles: m4[j, q, i] = mask[i, 128q + j]
    mm = np.asarray(mask[0, 0, :512, :512])
    m4 = (
        mm.T.reshape(4, 128, 512).transpose(1, 0, 2).astype(ml_dtypes.bfloat16)
    )  # [j, q, i]

    in_maps = []
    for c in range(NCORES):
        sl = slice(DQ * c, DQ * c + DQ)
        in_maps.append(
            {
                "xt": xt,
                "wq": np.ascontiguousarray(wq_p[:, sl]),
                "wk": np.ascontiguousarray(wk_p[:, sl]),
                "wv": np.ascontiguousarray(wv[:, sl]),
                "wo": np.ascontiguousarray(wo[sl, :]),
                "cc": cc,
                "ss": ss,
                "m4": m4,
            }
        )
    return in_maps


def kernel(**inputs):
    nc = _get_nc()
    in_maps = _host_inputs(
        inputs["x"], inputs["mask"], inputs["wq"], inputs["wk"],
        inputs["wv"], inputs["wo"],
    )
    res = run_bass_kernel_spmd(nc, in_maps, core_ids=list(range(NCORES)))
    out = res.results[0]["y"].astype(np.float64)
    for c in range(1, NCORES):
        out += res.results[c]["y"]
    return out.astype(np.float32)
